# revision 22
# baseline (speedup 1.0000x reference)
"""Trainium2 Bass kernel for nn_Attention_21878563405851.

Module: kv = x1 @ W_qk (k,v split); q = x2 @ W_v; 8-head attention
(dim_head=64); out @ W_out + b_out.  B=2, N=2048, DIM=512.

Sharding over 8 NeuronCores: core c -> batch b=c//4, query slice
qs = (c%4)*512 .. +512.  ZERO collectives: each core duplicates the
k/v projection for its whole batch locally (cheaper than the 20-40us
AllGathers a head-sharded variant needs) and computes its own
512-query slice of the output end to end.

All matmuls are bf16 (fp8 DoubleRow dots measured SLOWER: mixing
fp8-DR and bf16 instructions drops the whole PE phase to ~630ns/
matmul, and the fp8 drains add ~12us of DVE to the critical path).

Per core:
  1. x1[b]^T streams in as 16 piece-major [128,512] DMAs feeding a
     4-set k-projection SWEEP (piece-outer, set-mid, chunk-inner) so
     the PE consumes pieces at DMA arrival pace with no stalls and
     ALL k sets finish with the x1 transfer.  q projection + v
     projection (key-major v_ext with a ones column per head for the
     softmax denominator -- no PE transposes) follow at full speed.
     W_out/bias loads are deferred behind the priority weights,
  2. attention runs as a pure stream, one head at a time: per key-
     tile dots^T = k @ q^T ([128,512], K=64); per key-tile-pair one
     [128,1024] exp on ACT -> bf16 e tile -> attnv into [65,512]
     PSUM.  Software-pipelined (dots of pair p+1 emitted before attnv
     of pair p) so the in-order PE queue never waits on ACT,
  3. per-head normalization (reciprocal of the ones-row + DMA
     broadcast + DVE multiply) overlaps the next head,
  4. the final projection is spread through the attention phase:
     after heads 2c,2c+1 normalize, chunk c of y^T = W_out^T@attn_out
     runs as 4 matmuls into transient PSUM and accumulates into SBUF
     f32 via DVE (bias folded into the c=0 pass as a per-partition
     tensor_scalar add), so only chunk 3 + 4 output DMAs remain after
     the last head.  Host transposes/concatenates yT [512,512] f32.
"""

import sys

for _p in ("/opt/trn_rl_repo", "/root/.axon_site/_ro/trn_rl_repo"):
    if _p not in sys.path:
        sys.path.insert(0, _p)

import numpy as np
import ml_dtypes

import concourse.bass as bass
import concourse.mybir as mybir
from concourse import tile
from concourse.bacc import Bacc

B, N, DIM = 2, 2048, 512
HEADS, DH = 8, 64
INNER = HEADS * DH
SCALE = DH ** -0.5
NCORES = 8
NQ = 512             # queries per core
NKT = N // 128       # key tiles (16)
NPAIR = NKT // 2     # key-tile pairs (8)
NC_CHUNKS = DIM // 128
NSETS = 4            # head-pair sets (2 heads x 64 = 128 cols each)
VW = 66              # v_ext per-head stride (64 v cols + ones + pad)
VKT = HEADS * VW     # v_ext per-key-tile stride (528)

BF16 = mybir.dt.bfloat16
F32 = mybir.dt.float32


def build_program():
    nc = Bacc(None, num_devices=NCORES)

    # ---- external I/O (per core) ----
    x1T = nc.dram_tensor("x1T", [128, NC_CHUNKS * N], BF16, kind="ExternalInput")
    x2T = nc.dram_tensor("x2T", [128, NC_CHUNKS * NQ], BF16, kind="ExternalInput")
    wk = nc.dram_tensor("wk", [128, NSETS * NC_CHUNKS * 128], BF16, kind="ExternalInput")
    wq = nc.dram_tensor("wq", [128, NSETS * NC_CHUNKS * 128], BF16, kind="ExternalInput")
    wv = nc.dram_tensor("wv", [128, NC_CHUNKS * 512], BF16, kind="ExternalInput")
    wo = nc.dram_tensor("wo", [128, NC_CHUNKS * NC_CHUNKS * 128], BF16, kind="ExternalInput")
    bo = nc.dram_tensor("bo", [128, NC_CHUNKS], F32, kind="ExternalInput")
    yT = nc.dram_tensor("yT", [DIM, NQ], F32, kind="ExternalOutput")

    with tile.TileContext(nc) as tc:
        with (
            tc.tile_pool(name="xin", bufs=1) as xin,
            tc.tile_pool(name="wts", bufs=1) as wts,
            tc.tile_pool(name="kq", bufs=1) as kqp,
            tc.tile_pool(name="vext", bufs=1) as vextp,
            tc.tile_pool(name="et", bufs=4) as etp,
            tc.tile_pool(name="norm", bufs=2) as normp,
            tc.tile_pool(name="outp", bufs=1) as outp,
            tc.tile_pool(name="ysb", bufs=1) as ysbp,
            # PSUM: ps_pair = 3 rotating [128,1024] f32 slots (6 banks)
            # for proj / dots / final-y; ps_acc = 2 rotating [65,512]
            # attnv accumulators (2 banks).
            tc.tile_pool(name="ps_pair", bufs=3, space="PSUM") as psp,
            tc.tile_pool(name="ps_acc", bufs=2, space="PSUM") as ps_acc,
            tc.tile_pool(name="dram", bufs=1, space="DRAM") as dramp,
        ):
            # ---- loads.  gpsimd: wk, wq (k-sweep needs all of wk
            # early), then deferred wo/bo.  sync: x1 pieces, piece-
            # major to match the k-sweep consumption order.  scalar:
            # x2, wv.
            # all loads up front: DMA running concurrently with the PE
            # measurably slows matmuls (375 -> 457 -> 630 ns/mm as more
            # DMA overlaps compute), so everything must land before the
            # attention phase starts.
            wk_s = wts.tile([128, NSETS * NC_CHUNKS * 128], BF16, name="wk_s")
            SETW = NC_CHUNKS * 128
            nc.gpsimd.dma_start(wk_s[:, 0:SETW], wk[:, 0:SETW])
            x1_s = xin.tile([128, NC_CHUNKS * N], BF16, name="x1_s")
            for c in range(NC_CHUNKS):
                nc.sync.dma_start(
                    x1_s[:, c * N:(c + 1) * N], x1T[:, c * N:(c + 1) * N]
                )
            wq_s = wts.tile([128, NSETS * NC_CHUNKS * 128], BF16, name="wq_s")
            nc.gpsimd.dma_start(wq_s[:, 0:SETW], wq[:, 0:SETW])
            x2_s = xin.tile([128, NC_CHUNKS * NQ], BF16, name="x2_s")
            nc.scalar.dma_start(x2_s[:], x2T[:])
            wv_s = wts.tile([128, NC_CHUNKS * 512], BF16, name="wv_s")
            nc.scalar.dma_start(wv_s[:], wv[:])
            nc.gpsimd.dma_start(wk_s[:, SETW:], wk[:, SETW:])
            nc.gpsimd.dma_start(wq_s[:, SETW:], wq[:, SETW:])
            wo_s = wts.tile([128, NC_CHUNKS * NC_CHUNKS * 128], BF16, name="wo_s")
            nc.scalar.dma_start(wo_s[:], wo[:])
            bo_s = wts.tile([128, NC_CHUNKS], F32, name="bo_s")
            nc.gpsimd.dma_start(bo_s[:], bo[:])

            # persistent SBUF tensors
            kT = [
                kqp.tile([128, N], BF16, name=f"kT{s}") for s in range(NSETS)
            ]  # set s: heads 2s (rows 0:64), 2s+1 (rows 64:128), d-major
            qT = [
                kqp.tile([128, NQ], BF16, name=f"qT{s}") for s in range(NSETS)
            ]
            v_ext = vextp.tile([128, NKT * VKT], BF16, name="v_ext")
            outT = outp.tile([128, NC_CHUNKS * NQ], BF16, name="outT")
            y_sb = ysbp.tile([128, NC_CHUNKS * NQ], F32, name="y_sb")

            def k_proj(s):
                """kT[s] <- (W_qk k-cols for heads 2s,2s+1)^T @ x1[b]^T."""
                ts = [
                    psp.tile([128, 1024], F32, name=f"kp{s}{i}", tag="ps")
                    for i in range(2)
                ]
                for c in range(NC_CHUNKS):
                    for p in range(4):
                        nc.tensor.matmul(
                            ts[p // 2][:, (p % 2) * 512:(p % 2) * 512 + 512],
                            wk_s[:, (s * NC_CHUNKS + c) * 128:(s * NC_CHUNKS + c + 1) * 128],
                            x1_s[:, c * N + p * 512: c * N + (p + 1) * 512],
                            start=(c == 0),
                            stop=(c == NC_CHUNKS - 1),
                        )
                for i in range(2):
                    nc.vector.tensor_copy(
                        kT[s][:, i * 1024:(i + 1) * 1024], ts[i][:]
                    )

            def q_proj(s):
                ts = psp.tile([128, 1024], F32, name=f"qp{s}", tag="ps")
                for c in range(NC_CHUNKS):
                    nc.tensor.matmul(
                        ts[:, 0:512],
                        wq_s[:, (s * NC_CHUNKS + c) * 128:(s * NC_CHUNKS + c + 1) * 128],
                        x2_s[:, c * NQ:(c + 1) * NQ],
                        start=(c == 0),
                        stop=(c == NC_CHUNKS - 1),
                    )
                nc.vector.tensor_copy(qT[s][:], ts[:, 0:512])

            def v_proj():
                for pr in range(NPAIR):
                    ts = psp.tile([128, 1024], F32, name=f"vp{pr}", tag="ps")
                    for half in range(2):
                        kt = 2 * pr + half
                        for c in range(NC_CHUNKS):
                            nc.tensor.matmul(
                                ts[:, half * 512:(half + 1) * 512],
                                x1_s[:, c * N + kt * 128: c * N + (kt + 1) * 128],
                                wv_s[:, c * 512:(c + 1) * 512],
                                start=(c == 0),
                                stop=(c == NC_CHUNKS - 1),
                            )
                    nc.vector.tensor_copy(
                        v_ext[:, 2 * pr * VKT:(2 * pr + 2) * VKT].rearrange(
                            "p (two h w) -> p two h w", two=2, h=HEADS, w=VW
                        )[:, :, :, 0:DH],
                        ts[:].rearrange(
                            "p (two h w) -> p two h w", two=2, h=HEADS, w=DH
                        ),
                    )
                nc.vector.memset(
                    v_ext[:].rearrange(
                        "p (kt h w) -> p kt h w", kt=NKT, h=HEADS, w=VW
                    )[:, :, :, DH:DH + 1],
                    1.0,
                )

            k_proj(0)
            q_proj(0)
            v_proj()

            # ---- attention + spread final projection ----
            pending = None  # (head, pair, e_tile)
            accs = {}

            def y_chunk(c):
                """yT partial for outT chunk c (heads 2c,2c+1): 4
                matmuls into transient PSUM, DVE-accumulated into y_sb
                (bias folded into the c==0 pass)."""
                ts = [
                    psp.tile([128, 1024], F32, name=f"yc{c}{i}", tag="ps")
                    for i in range(2)
                ]
                for m in range(NC_CHUNKS):
                    nc.tensor.matmul(
                        ts[m // 2][:, (m % 2) * 512:(m % 2) * 512 + 512],
                        wo_s[:, (m * NC_CHUNKS + c) * 128:(m * NC_CHUNKS + c + 1) * 128],
                        outT[:, c * NQ:(c + 1) * NQ],
                    )
                if c == 0:
                    for m in range(NC_CHUNKS):
                        nc.vector.tensor_scalar_add(
                            y_sb[:, m * NQ:(m + 1) * NQ],
                            ts[m // 2][:, (m % 2) * 512:(m % 2) * 512 + 512],
                            bo_s[:, m:m + 1],
                        )
                else:
                    for i in range(2):
                        nc.vector.tensor_add(
                            y_sb[:, i * 1024:(i + 1) * 1024],
                            y_sb[:, i * 1024:(i + 1) * 1024],
                            ts[i][:],
                        )

            def emit_attnv(h, pr, e_t):
                acc = accs[h]
                for half in range(2):
                    kt = 2 * pr + half
                    nc.tensor.matmul(
                        acc[:, 0:512],
                        v_ext[:, kt * VKT + (h * VW): kt * VKT + (h * VW) + 65],
                        e_t[:, half * 512:(half + 1) * 512],
                        start=(kt == 0),
                        stop=(kt == NKT - 1),
                    )
                if pr == NPAIR - 1:
                    emit_norm(h)

            def emit_norm(h):
                acc = accs[h]
                s_s = normp.tile([1, NQ], F32, name="s_s", tag="s1")
                r_s = normp.tile([1, NQ], F32, name="r_s", tag="s2")
                rb_s = normp.tile([64, NQ], F32, name="rb_s", tag="rb")
                nc.vector.tensor_copy(s_s[:], acc[64:65, :])
                nc.vector.reciprocal_approx_fast(r_s[:], s_s[:])
                r_dram = dramp.tile([1, NQ], F32, name="r_dram", tag="r_dram", bufs=2)
                nc.sync.dma_start(r_dram[:], r_s[:])
                nc.sync.dma_start(rb_s[:], r_dram[0:1, :].broadcast_to([64, NQ]))
                m, lo = h // 2, (h % 2) * 64
                nc.vector.tensor_mul(
                    outT[lo:lo + 64, m * NQ:(m + 1) * NQ], acc[0:64, :], rb_s[:]
                )

            for h in range(HEADS):
                s, lo = h // 2, (h % 2) * 64
                # spread y chunks at clean head-top boundaries (norm of
                # heads 2c,2c+1 has been emitted by head 2c+2's start;
                # one extra head of margin avoids pipeline stalls)
                if h >= 3 and h % 2 == 1:
                    y_chunk((h - 3) // 2)
                accs[h] = ps_acc.tile([65, NQ], F32, name=f"acc{h}", tag="acc")
                for pr in range(NPAIR):
                    dt = psp.tile([128, 1024], F32, name="dt", tag="ps")
                    for half in range(2):
                        kt = 2 * pr + half
                        nc.tensor.matmul(
                            dt[:, half * 512:(half + 1) * 512],
                            kT[s][lo:lo + 64, kt * 128:(kt + 1) * 128],
                            qT[s][lo:lo + 64, :],
                        )
                    e_t = etp.tile([128, 1024], BF16, name="e_t", tag="e")
                    nc.scalar.activation(
                        e_t[:], dt[:],
                        mybir.ActivationFunctionType.Exp, scale=SCALE,
                    )
                    if pending is not None:
                        emit_attnv(*pending)
                    pending = (h, pr, e_t)
                # interleave remaining projections under ACT slack
                if h < NSETS - 1:
                    k_proj(h + 1)
                    q_proj(h + 1)
            emit_attnv(*pending)

            # ---- remaining y chunks + output.  (Spreading y chunks
            # MID-pipeline — via emit_norm inside the deferred attnv —
            # was measured much slower; head-top insertion is safe.)
            y_chunk(NC_CHUNKS - 1)
            for m in range(NC_CHUNKS):
                eng = nc.scalar if m % 2 == 0 else nc.sync
                eng.dma_start(
                    yT[m * 128:(m + 1) * 128, :], y_sb[:, m * NQ:(m + 1) * NQ]
                )

    nc.finalize()
    return nc


_NC_CACHE = None


def _get_program():
    global _NC_CACHE
    if _NC_CACHE is None:
        _NC_CACHE = build_program()
    return _NC_CACHE


def _img_chunks(a):
    """[DIM, cols] -> SBUF chunk image [128, NC_CHUNKS*cols]."""
    cols = a.shape[1]
    return np.ascontiguousarray(
        a.reshape(NC_CHUNKS, 128, cols).transpose(1, 0, 2).reshape(128, -1)
    )


def make_in_maps(x1, x2, W_qk, W_v, W_out, b_out):
    bf = ml_dtypes.bfloat16
    x1 = np.asarray(x1, np.float32)
    x2 = np.asarray(x2, np.float32)
    W_qk = np.asarray(W_qk, np.float32).astype(bf)
    W_v = np.asarray(W_v, np.float32).astype(bf)
    W_out = np.asarray(W_out, np.float32).astype(bf)
    b_out = np.asarray(b_out, np.float32)

    wk_img = np.ascontiguousarray(
        np.stack(
            [
                W_qk[c * 128:(c + 1) * 128, s * 128:(s + 1) * 128]
                for s in range(NSETS) for c in range(NC_CHUNKS)
            ], axis=1,
        ).reshape(128, -1)
    )
    wq_img = np.ascontiguousarray(
        np.stack(
            [
                W_v[c * 128:(c + 1) * 128, s * 128:(s + 1) * 128]
                for s in range(NSETS) for c in range(NC_CHUNKS)
            ], axis=1,
        ).reshape(128, -1)
    )
    wv_img = _img_chunks(W_qk[:, INNER:])
    wo_img = np.ascontiguousarray(
        np.stack(
            [
                W_out[c * 128:(c + 1) * 128, m * 128:(m + 1) * 128]
                for m in range(NC_CHUNKS) for c in range(NC_CHUNKS)
            ], axis=1,
        ).reshape(128, -1)
    )
    bo_img = np.ascontiguousarray(
        b_out.reshape(NC_CHUNKS, 128).T.astype(np.float32)
    )

    x1T_img = [
        _img_chunks(np.ascontiguousarray(x1[b].T).astype(bf)) for b in range(B)
    ]

    in_maps = []
    for c in range(NCORES):
        b, qi = c // 4, c % 4
        x2T_img = _img_chunks(
            np.ascontiguousarray(x2[b, qi * NQ:(qi + 1) * NQ, :].T).astype(bf)
        )
        in_maps.append(
            {
                "x1T": x1T_img[b],
                "x2T": x2T_img,
                "wk": wk_img,
                "wq": wq_img,
                "wv": wv_img,
                "wo": wo_img,
                "bo": bo_img,
            }
        )
    return in_maps


def assemble_output(results):
    y = np.empty((B, N, DIM), np.float32)
    for c in range(NCORES):
        b, qi = c // 4, c % 4
        y[b, qi * NQ:(qi + 1) * NQ, :] = results[c]["yT"].T
    return y


def kernel(x1, x2, W_qk, W_v, W_out, b_out):
    from concourse.bass_utils import run_bass_kernel_spmd

    nc = _get_program()
    in_maps = make_in_maps(x1, x2, W_qk, W_v, W_out, b_out)
    res = run_bass_kernel_spmd(nc, in_maps, list(range(NCORES)))
    return assemble_output(res.results)


# revision 24
# speedup vs baseline: 1.0276x; 1.0276x over previous
"""Trainium2 Bass kernel for nn_Attention_21878563405851.

Module: kv = x1 @ W_qk (k,v split); q = x2 @ W_v; 8-head attention
(dim_head=64); out @ W_out + b_out.  B=2, N=2048, DIM=512.

Sharding over 8 NeuronCores: core c -> batch b=c//4, query slice
qs = (c%4)*512 .. +512.  ZERO collectives: each core duplicates the
k/v projection for its whole batch locally (cheaper than the 20-40us
AllGathers a head-sharded variant needs) and computes its own
512-query slice of the output end to end.

All matmuls are bf16 (fp8 DoubleRow dots measured SLOWER: mixing
fp8-DR and bf16 instructions drops the whole PE phase to ~630ns/
matmul, and the fp8 drains add ~12us of DVE to the critical path).

Per core:
  1. x1[b]^T streams in as 16 piece-major [128,512] DMAs feeding a
     4-set k-projection SWEEP (piece-outer, set-mid, chunk-inner) so
     the PE consumes pieces at DMA arrival pace with no stalls and
     ALL k sets finish with the x1 transfer.  q projection + v
     projection (key-major v_ext with a ones column per head for the
     softmax denominator -- no PE transposes) follow at full speed.
     W_out/bias loads are deferred behind the priority weights,
  2. attention runs as a pure stream, one head at a time: per key-
     tile dots^T = k @ q^T ([128,512], K=64); per key-tile-pair one
     [128,1024] exp on ACT -> bf16 e tile -> attnv into [65,512]
     PSUM.  Software-pipelined (dots of pair p+1 emitted before attnv
     of pair p) so the in-order PE queue never waits on ACT,
  3. per-head normalization (reciprocal of the ones-row + DMA
     broadcast + DVE multiply) overlaps the next head,
  4. the final projection is spread through the attention phase:
     after heads 2c,2c+1 normalize, chunk c of y^T = W_out^T@attn_out
     runs as 4 matmuls into transient PSUM and accumulates into SBUF
     f32 via DVE (bias folded into the c=0 pass as a per-partition
     tensor_scalar add), so only chunk 3 + 4 output DMAs remain after
     the last head.  Host transposes/concatenates yT [512,512] f32.
"""

import sys

for _p in ("/opt/trn_rl_repo", "/root/.axon_site/_ro/trn_rl_repo"):
    if _p not in sys.path:
        sys.path.insert(0, _p)

import numpy as np
import ml_dtypes

import concourse.bass as bass
import concourse.mybir as mybir
from concourse import tile
from concourse.bacc import Bacc

B, N, DIM = 2, 2048, 512
HEADS, DH = 8, 64
INNER = HEADS * DH
SCALE = DH ** -0.5
NCORES = 8
NQ = 512             # queries per core
NKT = N // 128       # key tiles (16)
NPAIR = NKT // 2     # key-tile pairs (8)
NC_CHUNKS = DIM // 128
NSETS = 4            # head-pair sets (2 heads x 64 = 128 cols each)
VW = 66              # v_ext per-head stride (64 v cols + ones + pad)
VKT = HEADS * VW     # v_ext per-key-tile stride (528)

BF16 = mybir.dt.bfloat16
F32 = mybir.dt.float32


def build_program():
    nc = Bacc(None, num_devices=NCORES)

    # ---- external I/O (per core) ----
    x1T = nc.dram_tensor("x1T", [128, NC_CHUNKS * N], BF16, kind="ExternalInput")
    x2T = nc.dram_tensor("x2T", [128, NC_CHUNKS * NQ], BF16, kind="ExternalInput")
    wk = nc.dram_tensor("wk", [128, NSETS * NC_CHUNKS * 128], BF16, kind="ExternalInput")
    wq = nc.dram_tensor("wq", [128, NSETS * NC_CHUNKS * 128], BF16, kind="ExternalInput")
    wv = nc.dram_tensor("wv", [128, NC_CHUNKS * 512], BF16, kind="ExternalInput")
    wo = nc.dram_tensor("wo", [128, NC_CHUNKS * NC_CHUNKS * 128], BF16, kind="ExternalInput")
    bo = nc.dram_tensor("bo", [128, NC_CHUNKS], F32, kind="ExternalInput")
    yT = nc.dram_tensor("yT", [DIM, NQ], F32, kind="ExternalOutput")

    with tile.TileContext(nc) as tc:
        with (
            tc.tile_pool(name="xin", bufs=1) as xin,
            tc.tile_pool(name="wts", bufs=1) as wts,
            tc.tile_pool(name="kq", bufs=1) as kqp,
            tc.tile_pool(name="vext", bufs=1) as vextp,
            tc.tile_pool(name="et", bufs=4) as etp,
            tc.tile_pool(name="norm", bufs=2) as normp,
            tc.tile_pool(name="outp", bufs=1) as outp,
            tc.tile_pool(name="ysb", bufs=1) as ysbp,
            # PSUM: ps_pair = 3 rotating [128,1024] f32 slots (6 banks)
            # for proj / dots / final-y; ps_acc = 2 rotating [65,512]
            # attnv accumulators (2 banks).
            tc.tile_pool(name="ps_pair", bufs=3, space="PSUM") as psp,
            tc.tile_pool(name="ps_acc", bufs=2, space="PSUM") as ps_acc,
            tc.tile_pool(name="dram", bufs=1, space="DRAM") as dramp,
        ):
            # ---- loads.  gpsimd: wk, wq (k-sweep needs all of wk
            # early), then deferred wo/bo.  sync: x1 pieces, piece-
            # major to match the k-sweep consumption order.  scalar:
            # x2, wv.
            # all loads up front: DMA running concurrently with the PE
            # measurably slows matmuls (375 -> 457 -> 630 ns/mm as more
            # DMA overlaps compute), so everything must land before the
            # attention phase starts.
            wk_s = wts.tile([128, NSETS * NC_CHUNKS * 128], BF16, name="wk_s")
            SETW = NC_CHUNKS * 128
            nc.gpsimd.dma_start(wk_s[:, 0:SETW], wk[:, 0:SETW])
            x1_s = xin.tile([128, NC_CHUNKS * N], BF16, name="x1_s")
            # chunk 0 split into 4 pieces so the first k-proj matmul
            # gates on 128KB; later chunks stay whole (4KB DMA rows)
            for p in range(4):
                nc.sync.dma_start(
                    x1_s[:, p * 512:(p + 1) * 512], x1T[:, p * 512:(p + 1) * 512]
                )
            for c in range(1, NC_CHUNKS):
                nc.sync.dma_start(
                    x1_s[:, c * N:(c + 1) * N], x1T[:, c * N:(c + 1) * N]
                )
            wq_s = wts.tile([128, NSETS * NC_CHUNKS * 128], BF16, name="wq_s")
            nc.gpsimd.dma_start(wq_s[:, 0:SETW], wq[:, 0:SETW])
            x2_s = xin.tile([128, NC_CHUNKS * NQ], BF16, name="x2_s")
            nc.scalar.dma_start(x2_s[:], x2T[:])
            wv_s = wts.tile([128, NC_CHUNKS * 512], BF16, name="wv_s")
            nc.scalar.dma_start(wv_s[:], wv[:])
            nc.gpsimd.dma_start(wk_s[:, SETW:], wk[:, SETW:])
            nc.gpsimd.dma_start(wq_s[:, SETW:], wq[:, SETW:])
            wo_s = wts.tile([128, NC_CHUNKS * NC_CHUNKS * 128], BF16, name="wo_s")
            nc.scalar.dma_start(wo_s[:], wo[:])
            bo_s = wts.tile([128, NC_CHUNKS], F32, name="bo_s")
            nc.gpsimd.dma_start(bo_s[:], bo[:])

            # persistent SBUF tensors
            kT = [
                kqp.tile([128, N], BF16, name=f"kT{s}") for s in range(NSETS)
            ]  # set s: heads 2s (rows 0:64), 2s+1 (rows 64:128), d-major
            qT = [
                kqp.tile([128, NQ], BF16, name=f"qT{s}") for s in range(NSETS)
            ]
            v_ext = vextp.tile([128, NKT * VKT], BF16, name="v_ext")
            outT = outp.tile([128, NC_CHUNKS * NQ], BF16, name="outT")
            y_sb = ysbp.tile([128, NC_CHUNKS * NQ], F32, name="y_sb")

            def k_proj(s):
                """kT[s] <- (W_qk k-cols for heads 2s,2s+1)^T @ x1[b]^T."""
                ts = [
                    psp.tile([128, 1024], F32, name=f"kp{s}{i}", tag="ps")
                    for i in range(2)
                ]
                for c in range(NC_CHUNKS):
                    for p in range(4):
                        nc.tensor.matmul(
                            ts[p // 2][:, (p % 2) * 512:(p % 2) * 512 + 512],
                            wk_s[:, (s * NC_CHUNKS + c) * 128:(s * NC_CHUNKS + c + 1) * 128],
                            x1_s[:, c * N + p * 512: c * N + (p + 1) * 512],
                            start=(c == 0),
                            stop=(c == NC_CHUNKS - 1),
                        )
                for i in range(2):
                    nc.vector.tensor_copy(
                        kT[s][:, i * 1024:(i + 1) * 1024], ts[i][:]
                    )

            def q_proj(s):
                ts = psp.tile([128, 1024], F32, name=f"qp{s}", tag="ps")
                for c in range(NC_CHUNKS):
                    nc.tensor.matmul(
                        ts[:, 0:512],
                        wq_s[:, (s * NC_CHUNKS + c) * 128:(s * NC_CHUNKS + c + 1) * 128],
                        x2_s[:, c * NQ:(c + 1) * NQ],
                        start=(c == 0),
                        stop=(c == NC_CHUNKS - 1),
                    )
                nc.vector.tensor_copy(qT[s][:], ts[:, 0:512])

            def v_proj():
                for pr in range(NPAIR):
                    ts = psp.tile([128, 1024], F32, name=f"vp{pr}", tag="ps")
                    for half in range(2):
                        kt = 2 * pr + half
                        for c in range(NC_CHUNKS):
                            nc.tensor.matmul(
                                ts[:, half * 512:(half + 1) * 512],
                                x1_s[:, c * N + kt * 128: c * N + (kt + 1) * 128],
                                wv_s[:, c * 512:(c + 1) * 512],
                                start=(c == 0),
                                stop=(c == NC_CHUNKS - 1),
                            )
                    nc.vector.tensor_copy(
                        v_ext[:, 2 * pr * VKT:(2 * pr + 2) * VKT].rearrange(
                            "p (two h w) -> p two h w", two=2, h=HEADS, w=VW
                        )[:, :, :, 0:DH],
                        ts[:].rearrange(
                            "p (two h w) -> p two h w", two=2, h=HEADS, w=DH
                        ),
                    )
                nc.vector.memset(
                    v_ext[:].rearrange(
                        "p (kt h w) -> p kt h w", kt=NKT, h=HEADS, w=VW
                    )[:, :, :, DH:DH + 1],
                    1.0,
                )

            k_proj(0)
            q_proj(0)
            v_proj()

            # ---- attention + spread final projection ----
            pending = None  # (head, pair, e_tile)
            accs = {}

            def y_chunk(c):
                """yT partial for outT chunk c (heads 2c,2c+1): 4
                matmuls into transient PSUM, DVE-accumulated into y_sb
                (bias folded into the c==0 pass)."""
                ts = [
                    psp.tile([128, 1024], F32, name=f"yc{c}{i}", tag="ps")
                    for i in range(2)
                ]
                for m in range(NC_CHUNKS):
                    nc.tensor.matmul(
                        ts[m // 2][:, (m % 2) * 512:(m % 2) * 512 + 512],
                        wo_s[:, (m * NC_CHUNKS + c) * 128:(m * NC_CHUNKS + c + 1) * 128],
                        outT[:, c * NQ:(c + 1) * NQ],
                    )
                if c == 0:
                    for m in range(NC_CHUNKS):
                        nc.vector.tensor_scalar_add(
                            y_sb[:, m * NQ:(m + 1) * NQ],
                            ts[m // 2][:, (m % 2) * 512:(m % 2) * 512 + 512],
                            bo_s[:, m:m + 1],
                        )
                else:
                    for i in range(2):
                        nc.vector.tensor_add(
                            y_sb[:, i * 1024:(i + 1) * 1024],
                            y_sb[:, i * 1024:(i + 1) * 1024],
                            ts[i][:],
                        )

            def emit_attnv(h, pr, e_t):
                acc = accs[h]
                for half in range(2):
                    kt = 2 * pr + half
                    nc.tensor.matmul(
                        acc[:, 0:512],
                        v_ext[:, kt * VKT + (h * VW): kt * VKT + (h * VW) + 65],
                        e_t[:, half * 512:(half + 1) * 512],
                        start=(kt == 0),
                        stop=(kt == NKT - 1),
                    )
                if pr == NPAIR - 1:
                    emit_norm(h)

            def emit_norm(h):
                acc = accs[h]
                s_s = normp.tile([1, NQ], F32, name="s_s", tag="s1")
                r_s = normp.tile([1, NQ], F32, name="r_s", tag="s2")
                rb_s = normp.tile([64, NQ], F32, name="rb_s", tag="rb")
                nc.vector.tensor_copy(s_s[:], acc[64:65, :])
                nc.vector.reciprocal_approx_fast(r_s[:], s_s[:])
                r_dram = dramp.tile([1, NQ], F32, name="r_dram", tag="r_dram", bufs=2)
                nc.sync.dma_start(r_dram[:], r_s[:])
                nc.sync.dma_start(rb_s[:], r_dram[0:1, :].broadcast_to([64, NQ]))
                m, lo = h // 2, (h % 2) * 64
                nc.vector.tensor_mul(
                    outT[lo:lo + 64, m * NQ:(m + 1) * NQ], acc[0:64, :], rb_s[:]
                )

            for h in range(HEADS):
                s, lo = h // 2, (h % 2) * 64
                accs[h] = ps_acc.tile([65, NQ], F32, name=f"acc{h}", tag="acc")
                for pr in range(NPAIR):
                    dt = psp.tile([128, 1024], F32, name="dt", tag="ps")
                    for half in range(2):
                        kt = 2 * pr + half
                        nc.tensor.matmul(
                            dt[:, half * 512:(half + 1) * 512],
                            kT[s][lo:lo + 64, kt * 128:(kt + 1) * 128],
                            qT[s][lo:lo + 64, :],
                        )
                    e_t = etp.tile([128, 1024], BF16, name="e_t", tag="e")
                    nc.scalar.activation(
                        e_t[:], dt[:],
                        mybir.ActivationFunctionType.Exp, scale=SCALE,
                    )
                    if pending is not None:
                        emit_attnv(*pending)
                    pending = (h, pr, e_t)
                # interleave remaining projections under ACT slack
                if h < NSETS - 1:
                    k_proj(h + 1)
                    q_proj(h + 1)
            emit_attnv(*pending)

            # ---- final projection at the end (spreading y chunks into
            # the attention stream was measured MUCH slower: it breaks
            # the PSUM rotation every pair, the PE stalls and its clock
            # drops to 1.2GHz for the rest of the phase) ----
            for c in range(NC_CHUNKS):
                y_chunk(c)
            for m in range(NC_CHUNKS):
                nc.scalar.dma_start(
                    yT[m * 128:(m + 1) * 128, :], y_sb[:, m * NQ:(m + 1) * NQ]
                )

    nc.finalize()
    return nc


_NC_CACHE = None


def _get_program():
    global _NC_CACHE
    if _NC_CACHE is None:
        _NC_CACHE = build_program()
    return _NC_CACHE


def _img_chunks(a):
    """[DIM, cols] -> SBUF chunk image [128, NC_CHUNKS*cols]."""
    cols = a.shape[1]
    return np.ascontiguousarray(
        a.reshape(NC_CHUNKS, 128, cols).transpose(1, 0, 2).reshape(128, -1)
    )


def make_in_maps(x1, x2, W_qk, W_v, W_out, b_out):
    bf = ml_dtypes.bfloat16
    x1 = np.asarray(x1, np.float32)
    x2 = np.asarray(x2, np.float32)
    W_qk = np.asarray(W_qk, np.float32).astype(bf)
    W_v = np.asarray(W_v, np.float32).astype(bf)
    W_out = np.asarray(W_out, np.float32).astype(bf)
    b_out = np.asarray(b_out, np.float32)

    wk_img = np.ascontiguousarray(
        np.stack(
            [
                W_qk[c * 128:(c + 1) * 128, s * 128:(s + 1) * 128]
                for s in range(NSETS) for c in range(NC_CHUNKS)
            ], axis=1,
        ).reshape(128, -1)
    )
    wq_img = np.ascontiguousarray(
        np.stack(
            [
                W_v[c * 128:(c + 1) * 128, s * 128:(s + 1) * 128]
                for s in range(NSETS) for c in range(NC_CHUNKS)
            ], axis=1,
        ).reshape(128, -1)
    )
    wv_img = _img_chunks(W_qk[:, INNER:])
    wo_img = np.ascontiguousarray(
        np.stack(
            [
                W_out[c * 128:(c + 1) * 128, m * 128:(m + 1) * 128]
                for m in range(NC_CHUNKS) for c in range(NC_CHUNKS)
            ], axis=1,
        ).reshape(128, -1)
    )
    bo_img = np.ascontiguousarray(
        b_out.reshape(NC_CHUNKS, 128).T.astype(np.float32)
    )

    x1T_img = [
        _img_chunks(np.ascontiguousarray(x1[b].T).astype(bf)) for b in range(B)
    ]

    in_maps = []
    for c in range(NCORES):
        b, qi = c // 4, c % 4
        x2T_img = _img_chunks(
            np.ascontiguousarray(x2[b, qi * NQ:(qi + 1) * NQ, :].T).astype(bf)
        )
        in_maps.append(
            {
                "x1T": x1T_img[b],
                "x2T": x2T_img,
                "wk": wk_img,
                "wq": wq_img,
                "wv": wv_img,
                "wo": wo_img,
                "bo": bo_img,
            }
        )
    return in_maps


def assemble_output(results):
    y = np.empty((B, N, DIM), np.float32)
    for c in range(NCORES):
        b, qi = c // 4, c % 4
        y[b, qi * NQ:(qi + 1) * NQ, :] = results[c]["yT"].T
    return y


def kernel(x1, x2, W_qk, W_v, W_out, b_out):
    from concourse.bass_utils import run_bass_kernel_spmd

    nc = _get_program()
    in_maps = make_in_maps(x1, x2, W_qk, W_v, W_out, b_out)
    res = run_bass_kernel_spmd(nc, in_maps, list(range(NCORES)))
    return assemble_output(res.results)
